# revision 6
# baseline (speedup 1.0000x reference)
"""Top-1 MoE layer (BASE-layer style) on 8 Trainium2 NeuronCores.

Expert-parallel: core e holds expert e's weights. The host computes the
top-1 gating assignment (a tiny [T,E] matmul + argmax), dispatches each
expert's tokens to its core (this realizes the All2All of the reference
module), each core runs LN -> FF1 -> ReLU -> FF2 -> +residual over its
token batch, and the host scatters the per-expert outputs back into
token order.

Per-core device kernel (capacity C tokens, D=1024, F=4096):
  - LN in token-major layout via bn_stats/bn_aggr
  - PE-transpose of xn into D-major, LN affine fused into the eviction
  - MM1: hT[f,t] = relu(W1.T @ xnT + b1), bf16 matmul, b1+relu fused
    into the PSUM eviction on ScalarE
  - MM2: y[t,d] = hT.T @ W2 + (x + b2), residual add fused into the
    PSUM eviction on VectorE
Weights are cast to bf16 on the host; activations/LN/residual stay fp32.
"""

import math

import numpy as np
import ml_dtypes

import concourse.bass as bass
import concourse.tile as tile
from concourse import bacc, mybir
from concourse.bass_utils import run_bass_kernel_spmd
from concourse.masks import make_identity

E = 8
D = 1024
F = 4096
LN_EPS = 1e-5
P = 128
F32 = mybir.dt.float32
BF16 = mybir.dt.bfloat16

# set by test.py to get a profile
TRACE = False
TRACE_DIR = None
LAST_EXEC_TIME_NS = None
LAST_RESULTS = None

_program_cache = {}


def build_program(C: int):
    """Build the SPMD per-core Bass program for token capacity C (mult of 128)."""
    assert C % P == 0
    nc = bacc.Bacc(None, target_bir_lowering=False, debug=False)

    xe_d = nc.dram_tensor("xe", [C, D], F32, kind="ExternalInput")
    w1_d = nc.dram_tensor("w1", [D, F], BF16, kind="ExternalInput")
    w2_d = nc.dram_tensor("w2", [F, D], BF16, kind="ExternalInput")
    b1_d = nc.dram_tensor("b1", [F], F32, kind="ExternalInput")
    b2_d = nc.dram_tensor("b2", [D], F32, kind="ExternalInput")
    g_d = nc.dram_tensor("ln_g", [D], F32, kind="ExternalInput")
    bb_d = nc.dram_tensor("ln_b", [D], F32, kind="ExternalInput")
    ye_d = nc.dram_tensor("ye", [C, D], F32, kind="ExternalOutput")

    DO = D // P          # 8 d-tiles
    FO = F // P          # 32 f-tiles
    NDC = D // 512       # 2 output D chunks
    W1C = 512            # W1 f-chunk width
    NW1C = F // W1C      # 8 W1 chunks per block

    # token blocks: full 512-token blocks + remainder
    blocks = []
    t0 = 0
    while t0 < C:
        nt = min(4, (C - t0) // P)
        blocks.append((t0, nt))
        t0 += nt * P

    xe_r = xe_d[:].rearrange("(nt p) d -> p nt d", p=P)    # [128, C/128, D]
    ye_r = ye_d[:].rearrange("(nt p) d -> p nt d", p=P)
    w1_r = w1_d[:].rearrange("(do di) f -> di do f", di=P)  # [128, 8, F]
    w2_r = w2_d[:].rearrange("(fo fi) d -> fi fo d", fi=P)  # [128, 32, D]
    b1_r = b1_d[:].rearrange("(fo fi) -> fi fo", fi=P)      # [128, 32]
    g_r = g_d[:].rearrange("(do di) -> di do", di=P)        # [128, 8]
    bb_r = bb_d[:].rearrange("(do di) -> di do", di=P)

    with tile.TileContext(nc) as tc:
        with (
            tc.tile_pool(name="consts", bufs=1) as consts,
            tc.tile_pool(name="w2p", bufs=1) as w2p,
            tc.tile_pool(name="w1p", bufs=2) as w1p,
            tc.tile_pool(name="xp", bufs=1) as xp,
            tc.tile_pool(name="xnp", bufs=1) as xnp,
            tc.tile_pool(name="xtp", bufs=1) as xtp,
            tc.tile_pool(name="hp", bufs=1) as hp,
            tc.tile_pool(name="yp", bufs=2) as yp,
            tc.tile_pool(name="stat", bufs=4) as stat,
            tc.tile_pool(name="pst", bufs=2, space="PSUM") as pst,
            tc.tile_pool(name="psh", bufs=2, space="PSUM") as psh,
            tc.tile_pool(name="psy", bufs=2, space="PSUM") as psy,
        ):
            # ---- constants ----
            ident = consts.tile([P, P], BF16)
            make_identity(nc, ident)
            eps_t = consts.tile([P, 1], F32)
            nc.vector.memset(eps_t, LN_EPS)
            b1_t = consts.tile([P, FO], F32)
            nc.sync.dma_start(out=b1_t, in_=b1_r)
            g_t = consts.tile([P, DO], F32)
            nc.sync.dma_start(out=g_t, in_=g_r)
            bb_t = consts.tile([P, DO], F32)
            nc.sync.dma_start(out=bb_t, in_=bb_r)
            b2_t = consts.tile([P, D], F32)
            nc.sync.dma_start(
                out=b2_t,
                in_=b2_d[:].rearrange("(o d) -> o d", o=1).to_broadcast((P, D)),
            )

            # ---- resident W2 ----
            w2_t = w2p.tile([P, FO, D], BF16)
            for fo4 in range(4):
                nc.sync.dma_start(
                    out=w2_t[:, fo4 * 8:(fo4 + 1) * 8, :],
                    in_=w2_r[:, fo4 * 8:(fo4 + 1) * 8, :],
                )

            for (t0, nt_blk) in blocks:
                ntok = nt_blk * P
                nt0 = t0 // P

                x_t = xp.tile([P, nt_blk, D], F32, tag="x")
                nc.sync.dma_start(out=x_t, in_=xe_r[:, nt0:nt0 + nt_blk, :])

                xn_t = xnp.tile([P, nt_blk, D], BF16, tag="xn")
                for nt in range(nt_blk):
                    st = stat.tile([P, 2, 6], F32, tag="st")
                    for h in range(2):
                        nc.vector.bn_stats(
                            out=st[:, h, :], in_=x_t[:, nt, h * 512:(h + 1) * 512]
                        )
                    mv = stat.tile([P, 2], F32, tag="mv")
                    nc.vector.bn_aggr(out=mv, in_=st)
                    rstd = stat.tile([P, 1], F32, tag="rstd")
                    nc.scalar.activation(
                        out=rstd, in_=mv[:, 1:2],
                        func=mybir.ActivationFunctionType.Sqrt,
                        bias=eps_t, scale=1.0,
                    )
                    nc.vector.reciprocal(out=rstd, in_=rstd)
                    # xn = (x - mean) * rstd   (cast to bf16 on write)
                    nc.vector.tensor_scalar(
                        out=xn_t[:, nt, :], in0=x_t[:, nt, :],
                        scalar1=mv[:, 0:1], scalar2=rstd,
                        op0=mybir.AluOpType.subtract, op1=mybir.AluOpType.mult,
                    )
                    # after LN has consumed x, fold b2 into the residual
                    nc.vector.tensor_add(
                        out=x_t[:, nt, :], in0=x_t[:, nt, :], in1=b2_t
                    )

                # ---- transpose xn -> xnT [d_in, d_out, tok], LN affine fused ----
                xnT = xtp.tile([P, DO, ntok], BF16, tag="xnT")
                for nt in range(nt_blk):
                    for do in range(DO):
                        ps = pst.tile([P, P], BF16, tag="pst")
                        nc.tensor.transpose(
                            ps, xn_t[:, nt, do * P:(do + 1) * P], ident
                        )
                        # xnT = ps * g + b  (per-partition scalars in d-major)
                        nc.scalar.activation(
                            out=xnT[:, do, nt * P:(nt + 1) * P], in_=ps,
                            func=mybir.ActivationFunctionType.Identity,
                            bias=bb_t[:, do:do + 1], scale=g_t[:, do:do + 1],
                        )

                # ---- MM1: hT[f, t] = relu(W1.T @ xnT + b1) ----
                hT = hp.tile([P, FO, ntok], BF16, tag="hT")
                for c in range(NW1C):
                    w1c = w1p.tile([P, DO, W1C], BF16, tag="w1c")
                    nc.sync.dma_start(
                        out=w1c, in_=w1_r[:, :, c * W1C:(c + 1) * W1C]
                    )
                    for f in range(W1C // P):
                        fo = c * (W1C // P) + f
                        ph = psh.tile([P, ntok], F32, tag="ph")
                        for do in range(DO):
                            nc.tensor.matmul(
                                ph, w1c[:, do, f * P:(f + 1) * P], xnT[:, do, :],
                                start=(do == 0), stop=(do == DO - 1),
                            )
                        nc.scalar.activation(
                            out=hT[:, fo, :], in_=ph,
                            func=mybir.ActivationFunctionType.Relu,
                            bias=b1_t[:, fo:fo + 1], scale=1.0,
                        )

                # ---- MM2: y = hT.T @ W2 + (x + b2) ----
                for nt in range(nt_blk):
                    y_t = yp.tile([P, D], F32, tag="y")
                    for dc in range(NDC):
                        py = psy.tile([P, 512], F32, tag="py")
                        for fo in range(FO):
                            nc.tensor.matmul(
                                py, hT[:, fo, nt * P:(nt + 1) * P],
                                w2_t[:, fo, dc * 512:(dc + 1) * 512],
                                start=(fo == 0), stop=(fo == FO - 1),
                            )
                        nc.vector.tensor_add(
                            out=y_t[:, dc * 512:(dc + 1) * 512], in0=py,
                            in1=x_t[:, nt, dc * 512:(dc + 1) * 512],
                        )
                    nc.sync.dma_start(out=ye_r[:, nt0 + nt, :], in_=y_t)

    nc.compile()
    if not nc.is_finalized():
        nc.finalize()
    return nc


def kernel(input_features, centroids, ln_g, ln_b, W1, b1, W2, b2):
    global LAST_EXEC_TIME_NS
    x = np.asarray(input_features)
    S, B, _ = x.shape
    xt = np.ascontiguousarray(np.swapaxes(x, 0, 1).reshape(-1, D))  # [T, D]
    T = xt.shape[0]

    # host gating: tiny [T,E] matmul + argmax (same fp32 math / first-max
    # tie-break as the reference)
    logits = xt @ np.asarray(centroids, np.float32).T
    assign = np.argmax(logits, axis=-1)
    order = [np.nonzero(assign == e)[0] for e in range(E)]
    counts = [len(o) for o in order]
    C = max(P, int(math.ceil(max(counts) / P)) * P)

    bf = ml_dtypes.bfloat16
    W1b = np.asarray(W1).astype(bf)
    W2b = np.asarray(W2).astype(bf)
    in_maps = []
    for e in range(E):
        xe = np.zeros((C, D), np.float32)
        xe[:counts[e]] = xt[order[e]]
        in_maps.append({
            "xe": xe,
            "w1": np.ascontiguousarray(W1b[e]),
            "w2": np.ascontiguousarray(W2b[e]),
            "b1": np.asarray(b1[e], np.float32),
            "b2": np.asarray(b2[e], np.float32),
            "ln_g": np.asarray(ln_g[e], np.float32),
            "ln_b": np.asarray(ln_b[e], np.float32),
        })

    if C not in _program_cache:
        _program_cache[C] = build_program(C)
    nc = _program_cache[C]

    kw = {}
    if TRACE:
        kw = {"trace": True, "tmpdir": TRACE_DIR}
    res = run_bass_kernel_spmd(nc, in_maps, list(range(E)), **kw)
    LAST_EXEC_TIME_NS = res.exec_time_ns
    global LAST_RESULTS
    LAST_RESULTS = res

    out = np.empty((T, D), np.float32)
    for e in range(E):
        out[order[e]] = res.results[e]["ye"][:counts[e]]
    return np.ascontiguousarray(np.swapaxes(out.reshape(B, S, D), 0, 1))
